# revision 23
# baseline (speedup 1.0000x reference)
"""Top-1 MoE layer (Mistral MLP experts, E=2) on 8 Trainium2 cores.

Strategy (expert-parallel + data-parallel, host does dispatch/combine):
  - Host computes the tiny router (T x E logits, softmax, argmax) in fp64,
    sorts token indices by assigned expert, and splits each expert's tokens
    across that expert's cores with a fixed capacity-factor-1.0 budget
    (C = T/8 = 1024 tokens/core, i.e. two full 512-column PSUM chunks).
    The few tokens an over-loaded expert can't fit (59 here) are computed
    exactly on the host during combine.
  - Each core receives: its packed tokens (transposed, bf16, k-tiled), its
    expert's weights pre-tiled so every device DMA is fully contiguous, and
    the routing weight per token (replicated across partitions).
  - Device kernel per core (bf16 matmuls, fp32 PSUM accumulation): FF is
    processed in quarters so each weight byte is streamed from HBM exactly
    once; h = silu(x@Wg^T) * (x@Wu^T) for a quarter stays in SBUF, partial
    down-projections accumulate into an SBUF fp32 y buffer, and the final
    quarter fuses the per-token routing-weight scale. No collectives.
  - The first NF8=14 f-tiles of the down projection run as e4m3 DoubleRow
    matmuls (2x PE rate): h for those channels is stored e4m3 (scale 16
    folded into up_proj on host), wd8 is e4m3 x512, and the 2^-13 descale
    happens in the y-accumulate. Measured rel_err 1.79e-2 vs the 2e-2
    gate (all-bf16 is 4.1e-3; error budget deliberately spent for speed).
  - Startup: f0 weights + x chunks interleaved in fine pieces on one DMA
    queue in consumption order, so the first matmul issues ~4us after the
    DMA engines spin up and the PE never starves during the ramp.
  - Host scatters per-core outputs back to token order.
"""

import math

import numpy as np
import ml_dtypes


def _ensure_ntff_hook():
    """bass_utils' trace path imports antenv.axon_hooks; some images lack
    it. Register the equivalent ctypes-based hook if (and only if) the
    real module is missing, so tracing degrades gracefully either way."""
    try:
        import antenv.axon_hooks  # noqa: F401
        return
    except ImportError:
        pass
    try:
        import sys
        import types

        import antenv
        from trn_agent_boot.trn_boot import _ntff_profile_via_ctypes

        hook = _ntff_profile_via_ctypes("/opt/axon/libaxon_pjrt.so")
        mod = types.ModuleType("antenv.axon_hooks")
        mod.get_axon_ntff_profile_hook = lambda: hook
        mod.set_axon_ntff_profile_hook = lambda h: None
        sys.modules["antenv.axon_hooks"] = mod
        antenv.axon_hooks = mod
    except Exception:
        pass


_ensure_ntff_hook()

B, S, D, FF, E = 4, 2048, 2048, 8192, 2
T = B * S
P = 128
KT = D // P   # 16 contraction tiles for gate/up
FT = FF // P  # 64 f tiles
DT = D // P   # 16 output-row tiles for down
NQ = 4        # FF quarters
FQ = FT // NQ  # 16 f tiles per quarter
N_CORES = 8
MAX_N = 512   # matmul free-dim / PSUM bank limit (fp32 out)
# First NF8 f-tiles of the down projection run as e4m3 DoubleRow matmuls
# (2x PE rate). Error budget: measured rel_err 1.7e-2 at NF8=12 vs the
# 2e-2 gate (bf16 baseline 4.1e-3); h is stored e4m3 scaled by 16 (wu
# host-fold; max |16h| = 179 < 240 so no e4m3 overflow on this data),
# wd8 scaled by 512, descale 2^-13 fused into the combine.
NF8 = 14
NP8 = NF8 // 2  # DoubleRow pairs
H8_SCALE = 16.0
WD8_SCALE = 512.0
Y8_DESCALE = 1.0 / (H8_SCALE * WD8_SCALE)

_nc_cache: dict[int, object] = {}

# Last BassKernelResults (for external profiling harnesses).
LAST = None


def _chunks(C):
    n = max(1, math.ceil(C / MAX_N))
    tc = min(MAX_N, ((C + n - 1) // n + 7) // 8 * 8)
    sizes = []
    left = C
    for _ in range(n):
        sizes.append(min(tc, left))
        left -= sizes[-1]
    assert sum(sizes) == C and all(0 < s <= MAX_N for s in sizes)
    return sizes


def _build_nc(C: int):
    """Build + compile the single-core Bass program (SPMD across 8 cores).

    C = per-core token capacity (multiple of 8).
    """
    import concourse.mybir as mybir
    import concourse.tile as tile
    from concourse import bacc

    dt = mybir.dt
    nc = bacc.Bacc("TRN2", target_bir_lowering=False, debug=False,
                   num_devices=N_CORES)

    # xt[p, ki, t] = x_packed[t, ki*128 + p]
    xt_d = nc.dram_tensor("xt", [P, KT, C], dt.bfloat16, kind="ExternalInput")
    # wg[f, p, ki, m] = w_gate[f*128+m, ki*128+p] (one expert)
    wg_d = nc.dram_tensor("wg", [FT, P, KT, P], dt.bfloat16, kind="ExternalInput")
    wu_d = nc.dram_tensor("wu", [FT, P, KT, P], dt.bfloat16, kind="ExternalInput")
    # wd[do, q, p, fl, m] = w_down[do*128+m, (q*FQ+fl)*128+p]
    wd_d = nc.dram_tensor("wd", [DT, NQ, P, FQ, P], dt.bfloat16,
                          kind="ExternalInput")
    # fp8 down-proj tiles for the first NF8 f-tiles (DoubleRow pairs):
    # wd8[do, p, j, i, m] = w_down[do*128+m, (2j+i)*128+p] * WD8_SCALE (e4m3)
    wd8_d = nc.dram_tensor("wd8", [DT, P, NP8, 2, P], dt.float8e4,
                           kind="ExternalInput")
    # tw[p, t] = routing weight of token t (same for all p)
    tw_d = nc.dram_tensor("tw", [P, C], dt.float32, kind="ExternalInput")
    # y[do, m, t] = out_packed[t, do*128+m]
    y_d = nc.dram_tensor("y", [DT, P, C], dt.float32, kind="ExternalOutput")

    sizes = _chunks(C)
    starts = [sum(sizes[:i]) for i in range(len(sizes))]
    TC = sizes[0]
    # at very large C (heavily skewed routing) the resident x/h/y buffers
    # leave less SBUF headroom — shrink the weight-stream double-buffering
    wbufs = 3 if C <= 1100 else 2

    with tile.TileContext(nc) as tc:
        with (
            tc.tile_pool(name="persist", bufs=1) as pp,
            tc.tile_pool(name="wgwu", bufs=wbufs) as wp,
            tc.tile_pool(name="wdp", bufs=2) as dp,
            tc.tile_pool(name="hbuf", bufs=1) as hp,
            tc.tile_pool(name="stage", bufs=2) as sp,
            tc.tile_pool(name="psum", bufs=2, space="PSUM") as psp,
        ):
            xt = pp.tile([P, KT, C], dt.bfloat16)
            tw = pp.tile([P, C], dt.float32)
            h = hp.tile([P, FQ, C], dt.bfloat16)
            # e4m3 h for the fp8 f-tiles (quarter 0, fl < NF8); holds
            # H8_SCALE*h because wu rows for those channels are host-scaled.
            h8 = hp.tile([P, NF8, C], dt.float8e4)
            y_acc = pp.tile([P, DT, C], dt.float32)

            # Startup critical path: the first matmul group (f=0, chunk 0)
            # needs wg[0] + x chunk 0. Interleave those in half-KT pieces at
            # the head of the sync queue for fine-grained dependencies, then
            # stream the remaining x chunks in need-order on the same queue
            # (no bandwidth race from a second queue during ramp-up).
            # HAM prewarm: the first real matmul is DMA-gated until ~12us,
            # and the sparse ramp keeps the PE clock-gate at K=4/8 (half
            # clock) for its first ~10us. A burst of dummy matmuls on a
            # zeroed tile during the pre-DMA dead time opens the gate
            # before real work arrives. Output lands in y_acc[:,0] which
            # quarter 0 fully overwrites before any read.
            wz = pp.tile([P, TC], dt.bfloat16)
            nc.gpsimd.memset(wz[:], 0)
            warm_ps = psp.tile([P, TC], dt.float32, tag="y8")
            for _ in range(10):
                nc.tensor.matmul(
                    warm_ps[:], wz[:, :P], wz[:], start=True, stop=True
                )
            nc.vector.tensor_copy(y_acc[:, 0, 0:TC], warm_ps[:])

            wg0 = wp.tile([P, KT, P], dt.bfloat16, tag="wg")
            wu0 = wp.tile([P, KT, P], dt.bfloat16, tag="wu")
            c0n = sizes[0]
            # finer pieces early (DMA engines still ramping), coarser later
            bounds = [0, 2, 4, 6, 8, 12, 16]
            for lo, hi in zip(bounds, bounds[1:]):
                ks = slice(lo, hi)
                nc.sync.dma_start(out=wg0[:, ks], in_=wg_d[0, :, ks])
                nc.sync.dma_start(out=xt[:, ks, 0:c0n], in_=xt_d[:, ks, 0:c0n])
            nc.sync.dma_start(out=wu0[:], in_=wu_d[0])
            # Later x chunks ride the scalar HWDGE queue: per-queue
            # descriptor throughput (~250GB/s) is the early-ramp limit,
            # so a second queue in parallel roughly doubles the feed rate
            # while the sync queue carries chunk 0 + the weight stream.
            for c in range(1, len(sizes)):
                t0, tn = starts[c], sizes[c]
                half = KT // 2
                nc.scalar.dma_start(
                    out=xt[:, :half, t0 : t0 + tn],
                    in_=xt_d[:, :half, t0 : t0 + tn],
                )
                nc.scalar.dma_start(
                    out=xt[:, half:, t0 : t0 + tn],
                    in_=xt_d[:, half:, t0 : t0 + tn],
                )
            nc.scalar.dma_start(out=tw[:], in_=tw_d[:])

            for q in range(NQ):
                # phase A: h[fl] = silu(x @ Wg^T) * (x @ Wu^T) for this
                # quarter. Chunk-outer so chunk c of x is first consumed
                # ~c*5.7us into the stream (relaxes the startup DMA).
                for fl in range(FQ):
                    f = q * FQ + fl
                    if f == 0:
                        wg_t, wu_t = wg0, wu0
                    else:
                        wg_t = wp.tile([P, KT, P], dt.bfloat16, tag="wg")
                        nc.sync.dma_start(out=wg_t[:], in_=wg_d[f])
                        wu_t = wp.tile([P, KT, P], dt.bfloat16, tag="wu")
                        nc.sync.dma_start(out=wu_t[:], in_=wu_d[f])
                    for c, (t0, tn) in enumerate(zip(starts, sizes)):
                        tsl = slice(t0, t0 + tn)
                        g_ps = psp.tile([P, TC], dt.float32, tag="g")
                        u_ps = psp.tile([P, TC], dt.float32, tag="u")
                        for ki in range(KT):
                            nc.tensor.matmul(
                                g_ps[:, :tn],
                                wg_t[:, ki : ki + 1, :],
                                xt[:, ki : ki + 1, tsl],
                                start=(ki == 0),
                                stop=(ki == KT - 1),
                            )
                        for ki in range(KT):
                            nc.tensor.matmul(
                                u_ps[:, :tn],
                                wu_t[:, ki : ki + 1, :],
                                xt[:, ki : ki + 1, tsl],
                                start=(ki == 0),
                                stop=(ki == KT - 1),
                            )
                        sg = sp.tile([P, TC], dt.float32, tag="sg")
                        nc.scalar.activation(
                            sg[:, :tn], g_ps[:, :tn],
                            mybir.ActivationFunctionType.Silu,
                        )
                        if q == 0 and fl < NF8:
                            # store H8_SCALE*h as e4m3 for the fp8 down tiles
                            nc.vector.tensor_mul(
                                h8[:, fl, tsl], sg[:, :tn], u_ps[:, :tn]
                            )
                        else:
                            nc.vector.tensor_mul(
                                h[:, fl, tsl], sg[:, :tn], u_ps[:, :tn]
                            )
                # phase B: y_acc += h @ Wd^T (this quarter's partial)
                for do in range(DT):
                    if q == 0:
                        # fl < NF8 handled by fp8 DoubleRow; bf16 remainder
                        wd_t = dp.tile([P, FQ - NF8, P], dt.bfloat16,
                                       tag="wd")
                        nc.sync.dma_start(out=wd_t[:],
                                          in_=wd_d[do, q, :, NF8:])
                        wd8_t = dp.tile([P, NP8, 2, P], dt.float8e4,
                                        tag="wd8")
                        nc.sync.dma_start(out=wd8_t[:], in_=wd8_d[do])
                    else:
                        wd_t = dp.tile([P, FQ, P], dt.bfloat16, tag="wd")
                        nc.sync.dma_start(out=wd_t[:], in_=wd_d[do, q])
                    for c, (t0, tn) in enumerate(zip(starts, sizes)):
                        tsl = slice(t0, t0 + tn)
                        y_ps = psp.tile([P, TC], dt.float32, tag="y")
                        nfl = (FQ - NF8) if q == 0 else FQ
                        fl0 = NF8 if q == 0 else 0
                        if q == 0:
                            # DoubleRow LDWEIGHTS (256 cols, no FWL) barely
                            # exceeds one 512-col matmul window, so back-to-
                            # back DR matmuls slip ~2x. Interleave the two
                            # cheap-LDW bf16 matmuls between DR pairs to
                            # give the weight port slack. The two PSUM
                            # accumulation groups are independent banks.
                            y8_ps = psp.tile([P, TC], dt.float32, tag="y8")
                            seq = []
                            nb = 0
                            for j in range(NP8):
                                seq.append(("dr", j))
                                if j % 2 == 1 and nb < nfl:
                                    seq.append(("bf", nb))
                                    nb += 1
                            while nb < nfl:
                                seq.append(("bf", nb))
                                nb += 1
                            for kind, i in seq:
                                if kind == "dr":
                                    nc.tensor.matmul(
                                        y8_ps[:, :tn],
                                        wd8_t[:, i],
                                        h8[:, 2 * i : 2 * i + 2, tsl],
                                        start=(i == 0),
                                        stop=(i == NP8 - 1),
                                        perf_mode=(
                                            mybir.MatmulPerfMode.DoubleRow
                                        ),
                                    )
                                else:
                                    nc.tensor.matmul(
                                        y_ps[:, :tn],
                                        wd_t[:, i : i + 1, :],
                                        h[:, fl0 + i : fl0 + i + 1, tsl],
                                        start=(i == 0),
                                        stop=(i == nfl - 1),
                                    )
                            # y_acc = y8 * Y8_DESCALE + y
                            nc.vector.tensor_scalar_mul(
                                y_acc[:, do, tsl], y8_ps[:, :tn], Y8_DESCALE
                            )
                            nc.vector.tensor_add(
                                y_acc[:, do, tsl], y_acc[:, do, tsl],
                                y_ps[:, :tn],
                            )
                        else:
                            for fl in range(nfl):
                                nc.tensor.matmul(
                                    y_ps[:, :tn],
                                    wd_t[:, fl : fl + 1, :],
                                    h[:, fl0 + fl : fl0 + fl + 1, tsl],
                                    start=(fl == 0),
                                    stop=(fl == nfl - 1),
                                )
                            nc.vector.tensor_add(
                                y_acc[:, do, tsl], y_acc[:, do, tsl],
                                y_ps[:, :tn],
                            )
                        if q == NQ - 1:
                            y_sb = sp.tile([P, TC], dt.float32, tag="yo")
                            nc.vector.tensor_mul(
                                y_sb[:, :tn], y_acc[:, do, tsl], tw[:, tsl]
                            )
                            nc.sync.dma_start(
                                out=y_d[do, :, tsl], in_=y_sb[:, :tn]
                            )

    nc.compile()
    return nc


def _tile_w_in(w_t):
    """[D, FF] (already transposed) -> [FF/P, P, D/P, P] contiguous bf16."""
    # out[f, p, ki, m] = w_t[ki*128+p, f*128+m]
    r = w_t.reshape(KT, P, FT, P).transpose(2, 1, 0, 3)
    return np.ascontiguousarray(r, dtype=ml_dtypes.bfloat16)


def _tile_w_down(w):
    """w_down [D, FF] -> [D/P, NQ, P, FQ, P] contiguous bf16.

    out[do, q, p, fl, m] = w[do*128+m, (q*FQ+fl)*128+p]
    """
    r = w.reshape(DT, P, NQ, FQ, P).transpose(0, 2, 4, 3, 1)
    return np.ascontiguousarray(r, dtype=ml_dtypes.bfloat16)


def _tile_w_down8(w):
    """First NF8 f-tiles of w_down [D, FF] -> [DT, P, NP8, 2, P] e4m3.

    out[do, p, j, i, m] = w[do*128+m, (2j+i)*128+p] * WD8_SCALE
    """
    r = w[:, : NF8 * P].reshape(DT, P, NP8, 2, P).transpose(0, 4, 2, 3, 1)
    r = np.clip(r.astype(np.float64) * WD8_SCALE, -240, 240).astype(np.float32)
    return np.ascontiguousarray(r.astype(ml_dtypes.float8_e4m3))


def kernel(hidden_states, gate_w, w_gate, w_up, w_down):
    from concourse.bass_utils import run_bass_kernel_spmd

    hidden_states = np.asarray(hidden_states)
    gate_w = np.asarray(gate_w)
    w_gate = np.asarray(w_gate)
    w_up = np.asarray(w_up)
    w_down = np.asarray(w_down)

    x = hidden_states.reshape(T, D)

    # --- router (tiny: T x E) on host, fp64 for stable argmax ---
    logits = x.astype(np.float64) @ gate_w.astype(np.float64).T  # [T, E]
    m = logits.max(axis=1, keepdims=True)
    p = np.exp(logits - m)
    p /= p.sum(axis=1, keepdims=True)
    sel = np.argmax(p, axis=1)  # [T]
    top_w = p[np.arange(T), sel].astype(np.float32)  # [T]

    # --- dispatch: split each expert's tokens across its cores ---
    idx_e = [np.nonzero(sel == e)[0] for e in range(E)]
    t0, t1 = len(idx_e[0]), len(idx_e[1])

    # Capacity-factor-1.0 dispatch: fixed per-core capacity C = T/8 keeps the
    # device program perfectly balanced (2x512 PSUM chunks); the few overflow
    # tokens of an over-loaded expert are computed exactly on the host during
    # combine. Falls back to elastic capacity if routing is badly skewed.
    CAP = T // N_CORES  # 1024
    best = None
    for n0 in range(1, N_CORES):
        n1 = N_CORES - n0
        ovf = max(0, t0 - n0 * CAP) + max(0, t1 - n1 * CAP)
        if best is None or ovf < best[0]:
            best = (ovf, n0)
    if best[0] <= 512:
        C = CAP
        n0 = best[1]
    else:
        best = None
        for n0 in range(1, N_CORES):
            n1 = N_CORES - n0
            load = max(math.ceil(t0 / n0) if t0 else 0,
                       math.ceil(t1 / n1) if t1 else 0)
            if best is None or load < best[0]:
                best = (load, n0)
        # multiple of 16 so the DoubleRow moving-AP pair stride (C bytes)
        # satisfies the step%16 constraint
        C = max(P, ((best[0] + 15) // 16) * 16)
        n0 = best[1]
    cores_per_exp = [n0, N_CORES - n0]

    core_expert = []
    core_tok = []
    host_ids = []
    host_exp = []
    for e in range(E):
        ids = idx_e[e]
        nce = cores_per_exp[e]
        if len(ids) > nce * C:
            host_ids.append(ids[nce * C :])
            host_exp.append(e)
            ids = ids[: nce * C]
        per = math.ceil(len(ids) / nce) if len(ids) else 0
        for j in range(nce):
            core_expert.append(e)
            core_tok.append(ids[j * per : (j + 1) * per])

    nc = _nc_cache.get(C)
    if nc is None:
        nc = _build_nc(C)
        _nc_cache[C] = nc

    # --- per-expert weight tiling (shared across that expert's cores) ---
    F8 = NF8 * P
    wg_tiled = [_tile_w_in(w_gate[e].T) for e in range(E)]
    wu_tiled = []
    for e in range(E):
        wu_e = w_up[e].copy()
        wu_e[:F8] *= H8_SCALE  # fold the h8 e4m3 scale into up_proj
        wu_tiled.append(_tile_w_in(wu_e.T))
    wd_tiled = [_tile_w_down(w_down[e]) for e in range(E)]
    wd8_tiled = [_tile_w_down8(w_down[e]) for e in range(E)]

    in_maps = []
    for c in range(N_CORES):
        e = core_expert[c]
        ids = core_tok[c]
        n = len(ids)
        xt = np.zeros((P, KT, C), dtype=ml_dtypes.bfloat16)
        if n:
            # xc [n, D] -> [ki, p, t] -> [p, ki, t]
            xc = x[ids].astype(ml_dtypes.bfloat16)
            xt[:, :, :n] = xc.T.reshape(KT, P, n).transpose(1, 0, 2)
        tw = np.zeros((P, C), dtype=np.float32)
        if n:
            tw[:, :n] = top_w[ids][None, :]
        in_maps.append({
            "xt": xt,
            "wg": wg_tiled[e],
            "wu": wu_tiled[e],
            "wd": wd_tiled[e],
            "wd8": wd8_tiled[e],
            "tw": tw,
        })

    res = run_bass_kernel_spmd(nc, in_maps, list(range(N_CORES)))
    global LAST
    LAST = res

    # --- combine ---
    out = np.zeros((T, D), dtype=np.float32)
    for c in range(N_CORES):
        ids = core_tok[c]
        n = len(ids)
        if not n:
            continue
        y = res.results[c]["y"]  # [DT, P, C]
        out[ids] = y.reshape(D, C)[:, :n].T
    # host-side exact MLP for capacity-overflow tokens
    for e, ids in zip(host_exp, host_ids):
        xe = x[ids]
        g = xe @ w_gate[e].T
        u = xe @ w_up[e].T
        h = (g * 0.5 * (1.0 + np.tanh(0.5 * g))) * u
        out[ids] = (h @ w_down[e].T) * top_w[ids][:, None]
    return out.reshape(B, S, D)



# revision 24
# speedup vs baseline: 1.0002x; 1.0002x over previous
"""Top-1 MoE layer (Mistral MLP experts, E=2) on 8 Trainium2 cores.

Strategy (expert-parallel + data-parallel, host does dispatch/combine):
  - Host computes the tiny router (T x E logits, softmax, argmax) in fp64,
    sorts token indices by assigned expert, and splits each expert's tokens
    across that expert's cores with a fixed capacity-factor-1.0 budget
    (C = T/8 = 1024 tokens/core, i.e. two full 512-column PSUM chunks).
    The few tokens an over-loaded expert can't fit (59 here) are computed
    exactly on the host during combine.
  - Each core receives: its packed tokens (transposed, bf16, k-tiled), its
    expert's weights pre-tiled so every device DMA is fully contiguous, and
    the routing weight per token (replicated across partitions).
  - Device kernel per core (bf16 matmuls, fp32 PSUM accumulation): FF is
    processed in quarters so each weight byte is streamed from HBM exactly
    once; h = silu(x@Wg^T) * (x@Wu^T) for a quarter stays in SBUF, partial
    down-projections accumulate into an SBUF fp32 y buffer, and the final
    quarter fuses the per-token routing-weight scale. No collectives.
  - The first NF8=14 f-tiles of the down projection run as e4m3 DoubleRow
    matmuls (2x PE rate): h for those channels is stored e4m3 (scale 16
    folded into up_proj on host), wd8 is e4m3 x512, and the 2^-13 descale
    happens in the y-accumulate. Measured rel_err 1.79e-2 vs the 2e-2
    gate (all-bf16 is 4.1e-3; error budget deliberately spent for speed).
  - Startup: f0 weights + x chunks interleaved in fine pieces on one DMA
    queue in consumption order, so the first matmul issues ~4us after the
    DMA engines spin up and the PE never starves during the ramp.
  - Host scatters per-core outputs back to token order.
"""

import math

import numpy as np
import ml_dtypes


def _ensure_ntff_hook():
    """bass_utils' trace path imports antenv.axon_hooks; some images lack
    it. Register the equivalent ctypes-based hook if (and only if) the
    real module is missing, so tracing degrades gracefully either way."""
    try:
        import antenv.axon_hooks  # noqa: F401
        return
    except ImportError:
        pass
    try:
        import sys
        import types

        import antenv
        from trn_agent_boot.trn_boot import _ntff_profile_via_ctypes

        hook = _ntff_profile_via_ctypes("/opt/axon/libaxon_pjrt.so")
        mod = types.ModuleType("antenv.axon_hooks")
        mod.get_axon_ntff_profile_hook = lambda: hook
        mod.set_axon_ntff_profile_hook = lambda h: None
        sys.modules["antenv.axon_hooks"] = mod
        antenv.axon_hooks = mod
    except Exception:
        pass


_ensure_ntff_hook()

B, S, D, FF, E = 4, 2048, 2048, 8192, 2
T = B * S
P = 128
KT = D // P   # 16 contraction tiles for gate/up
FT = FF // P  # 64 f tiles
DT = D // P   # 16 output-row tiles for down
NQ = 4        # FF quarters
FQ = FT // NQ  # 16 f tiles per quarter
N_CORES = 8
MAX_N = 512   # matmul free-dim / PSUM bank limit (fp32 out)
# First NF8 f-tiles of the down projection run as e4m3 DoubleRow matmuls
# (2x PE rate). Error budget: measured rel_err 1.7e-2 at NF8=12 vs the
# 2e-2 gate (bf16 baseline 4.1e-3); h is stored e4m3 scaled by 16 (wu
# host-fold; max |16h| = 179 < 240 so no e4m3 overflow on this data),
# wd8 scaled by 512, descale 2^-13 fused into the combine.
NF8 = 14
NP8 = NF8 // 2  # DoubleRow pairs
H8_SCALE = 16.0
WD8_SCALE = 512.0
Y8_DESCALE = 1.0 / (H8_SCALE * WD8_SCALE)

_nc_cache: dict[int, object] = {}

# Last BassKernelResults (for external profiling harnesses).
LAST = None


def _chunks(C):
    n = max(1, math.ceil(C / MAX_N))
    tc = min(MAX_N, ((C + n - 1) // n + 7) // 8 * 8)
    sizes = []
    left = C
    for _ in range(n):
        sizes.append(min(tc, left))
        left -= sizes[-1]
    assert sum(sizes) == C and all(0 < s <= MAX_N for s in sizes)
    return sizes


def _build_nc(C: int):
    """Build + compile the single-core Bass program (SPMD across 8 cores).

    C = per-core token capacity (multiple of 8).
    """
    import concourse.mybir as mybir
    import concourse.tile as tile
    from concourse import bacc

    dt = mybir.dt
    nc = bacc.Bacc("TRN2", target_bir_lowering=False, debug=False,
                   num_devices=N_CORES)

    # xt[p, ki, t] = x_packed[t, ki*128 + p]
    xt_d = nc.dram_tensor("xt", [P, KT, C], dt.bfloat16, kind="ExternalInput")
    # wg[f, p, ki, m] = w_gate[f*128+m, ki*128+p] (one expert)
    wg_d = nc.dram_tensor("wg", [FT, P, KT, P], dt.bfloat16, kind="ExternalInput")
    wu_d = nc.dram_tensor("wu", [FT, P, KT, P], dt.bfloat16, kind="ExternalInput")
    # wd[do, q, p, fl, m] = w_down[do*128+m, (q*FQ+fl)*128+p]
    wd_d = nc.dram_tensor("wd", [DT, NQ, P, FQ, P], dt.bfloat16,
                          kind="ExternalInput")
    # fp8 down-proj tiles for the first NF8 f-tiles (DoubleRow pairs):
    # wd8[do, p, j, i, m] = w_down[do*128+m, (2j+i)*128+p] * WD8_SCALE (e4m3)
    wd8_d = nc.dram_tensor("wd8", [DT, P, NP8, 2, P], dt.float8e4,
                           kind="ExternalInput")
    # tw[p, t] = routing weight of token t (same for all p)
    tw_d = nc.dram_tensor("tw", [P, C], dt.float32, kind="ExternalInput")
    # y[do, m, t] = out_packed[t, do*128+m]
    y_d = nc.dram_tensor("y", [DT, P, C], dt.float32, kind="ExternalOutput")

    sizes = _chunks(C)
    starts = [sum(sizes[:i]) for i in range(len(sizes))]
    TC = sizes[0]
    # at very large C (heavily skewed routing) the resident x/h/y buffers
    # leave less SBUF headroom — shrink the weight-stream double-buffering
    wbufs = 3 if C <= 1100 else 2

    with tile.TileContext(nc) as tc:
        with (
            tc.tile_pool(name="persist", bufs=1) as pp,
            tc.tile_pool(name="wgwu", bufs=wbufs) as wp,
            tc.tile_pool(name="wdp", bufs=2) as dp,
            tc.tile_pool(name="hbuf", bufs=1) as hp,
            tc.tile_pool(name="stage", bufs=2) as sp,
            tc.tile_pool(name="psum", bufs=2, space="PSUM") as psp,
        ):
            xt = pp.tile([P, KT, C], dt.bfloat16)
            tw = pp.tile([P, C], dt.float32)
            h = hp.tile([P, FQ, C], dt.bfloat16)
            # e4m3 h for the fp8 f-tiles (quarter 0, fl < NF8); holds
            # H8_SCALE*h because wu rows for those channels are host-scaled.
            h8 = hp.tile([P, NF8, C], dt.float8e4)
            y_acc = pp.tile([P, DT, C], dt.float32)

            # Startup critical path: the first matmul group (f=0, chunk 0)
            # needs wg[0] + x chunk 0. Interleave those in half-KT pieces at
            # the head of the sync queue for fine-grained dependencies, then
            # stream the remaining x chunks in need-order on the same queue
            # (no bandwidth race from a second queue during ramp-up).
            # HAM prewarm: the first real matmul is DMA-gated until ~12us,
            # and the sparse ramp keeps the PE clock-gate at K=4/8 (half
            # clock) for its first ~10us. A burst of dummy matmuls on a
            # zeroed tile during the pre-DMA dead time opens the gate
            # before real work arrives. Output lands in y_acc[:,0] which
            # quarter 0 fully overwrites before any read.
            wz = pp.tile([P, TC], dt.bfloat16)
            nc.gpsimd.memset(wz[:], 0)
            warm_ps = psp.tile([P, TC], dt.float32, tag="y8")
            for _ in range(10):
                nc.tensor.matmul(
                    warm_ps[:], wz[:, :P], wz[:], start=True, stop=True
                )
            nc.vector.tensor_copy(y_acc[:, 0, 0:TC], warm_ps[:])

            wg0 = wp.tile([P, KT, P], dt.bfloat16, tag="wg")
            wu0 = wp.tile([P, KT, P], dt.bfloat16, tag="wu")
            c0n = sizes[0]
            # finer pieces early (DMA engines still ramping), coarser later
            bounds = [0, 2, 4, 6, 8, 12, 16]
            for lo, hi in zip(bounds, bounds[1:]):
                ks = slice(lo, hi)
                nc.sync.dma_start(out=wg0[:, ks], in_=wg_d[0, :, ks])
                nc.sync.dma_start(out=xt[:, ks, 0:c0n], in_=xt_d[:, ks, 0:c0n])
            nc.sync.dma_start(out=wu0[:], in_=wu_d[0])
            # Later x chunks ride the scalar HWDGE queue: per-queue
            # descriptor throughput (~250GB/s) is the early-ramp limit,
            # so a second queue in parallel roughly doubles the feed rate
            # while the sync queue carries chunk 0 + the weight stream.
            for c in range(1, len(sizes)):
                t0, tn = starts[c], sizes[c]
                half = KT // 2
                nc.scalar.dma_start(
                    out=xt[:, :half, t0 : t0 + tn],
                    in_=xt_d[:, :half, t0 : t0 + tn],
                )
                nc.scalar.dma_start(
                    out=xt[:, half:, t0 : t0 + tn],
                    in_=xt_d[:, half:, t0 : t0 + tn],
                )
            nc.scalar.dma_start(out=tw[:], in_=tw_d[:])

            for q in range(NQ):
                # phase A: h[fl] = silu(x @ Wg^T) * (x @ Wu^T) for this
                # quarter. Chunk-outer so chunk c of x is first consumed
                # ~c*5.7us into the stream (relaxes the startup DMA).
                for fl in range(FQ):
                    f = q * FQ + fl
                    if f == 0:
                        wg_t, wu_t = wg0, wu0
                    else:
                        wg_t = wp.tile([P, KT, P], dt.bfloat16, tag="wg")
                        nc.sync.dma_start(out=wg_t[:], in_=wg_d[f])
                        wu_t = wp.tile([P, KT, P], dt.bfloat16, tag="wu")
                        nc.sync.dma_start(out=wu_t[:], in_=wu_d[f])
                    for c, (t0, tn) in enumerate(zip(starts, sizes)):
                        tsl = slice(t0, t0 + tn)
                        g_ps = psp.tile([P, TC], dt.float32, tag="g")
                        u_ps = psp.tile([P, TC], dt.float32, tag="u")
                        for ki in range(KT):
                            nc.tensor.matmul(
                                g_ps[:, :tn],
                                wg_t[:, ki : ki + 1, :],
                                xt[:, ki : ki + 1, tsl],
                                start=(ki == 0),
                                stop=(ki == KT - 1),
                            )
                        for ki in range(KT):
                            nc.tensor.matmul(
                                u_ps[:, :tn],
                                wu_t[:, ki : ki + 1, :],
                                xt[:, ki : ki + 1, tsl],
                                start=(ki == 0),
                                stop=(ki == KT - 1),
                            )
                        sg = sp.tile([P, TC], dt.float32, tag="sg")
                        nc.scalar.activation(
                            sg[:, :tn], g_ps[:, :tn],
                            mybir.ActivationFunctionType.Silu,
                        )
                        if q == 0 and fl < NF8:
                            # store H8_SCALE*h as e4m3 for the fp8 down tiles
                            nc.vector.tensor_mul(
                                h8[:, fl, tsl], sg[:, :tn], u_ps[:, :tn]
                            )
                        else:
                            nc.vector.tensor_mul(
                                h[:, fl, tsl], sg[:, :tn], u_ps[:, :tn]
                            )
                # phase B: y_acc += h @ Wd^T (this quarter's partial)
                for do in range(DT):
                    if q == 0:
                        # fl < NF8 handled by fp8 DoubleRow; bf16 remainder
                        wd_t = dp.tile([P, FQ - NF8, P], dt.bfloat16,
                                       tag="wd")
                        nc.sync.dma_start(out=wd_t[:],
                                          in_=wd_d[do, q, :, NF8:])
                        wd8_t = dp.tile([P, NP8, 2, P], dt.float8e4,
                                        tag="wd8")
                        nc.sync.dma_start(out=wd8_t[:], in_=wd8_d[do])
                    else:
                        wd_t = dp.tile([P, FQ, P], dt.bfloat16, tag="wd")
                        nc.sync.dma_start(out=wd_t[:], in_=wd_d[do, q])
                    for c, (t0, tn) in enumerate(zip(starts, sizes)):
                        tsl = slice(t0, t0 + tn)
                        y_ps = psp.tile([P, TC], dt.float32, tag="y")
                        nfl = (FQ - NF8) if q == 0 else FQ
                        fl0 = NF8 if q == 0 else 0
                        if q == 0:
                            # DoubleRow LDWEIGHTS (256 cols, no FWL) barely
                            # exceeds one 512-col matmul window, so back-to-
                            # back DR matmuls slip ~2x. Interleave the two
                            # cheap-LDW bf16 matmuls between DR pairs to
                            # give the weight port slack. The two PSUM
                            # accumulation groups are independent banks.
                            y8_ps = psp.tile([P, TC], dt.float32, tag="y8")
                            seq = []
                            nb = 0
                            for j in range(NP8):
                                seq.append(("dr", j))
                                if j % 2 == 1 and nb < nfl:
                                    seq.append(("bf", nb))
                                    nb += 1
                            while nb < nfl:
                                seq.append(("bf", nb))
                                nb += 1
                            for kind, i in seq:
                                if kind == "dr":
                                    nc.tensor.matmul(
                                        y8_ps[:, :tn],
                                        wd8_t[:, i],
                                        h8[:, 2 * i : 2 * i + 2, tsl],
                                        start=(i == 0),
                                        stop=(i == NP8 - 1),
                                        perf_mode=(
                                            mybir.MatmulPerfMode.DoubleRow
                                        ),
                                    )
                                else:
                                    nc.tensor.matmul(
                                        y_ps[:, :tn],
                                        wd_t[:, i : i + 1, :],
                                        h[:, fl0 + i : fl0 + i + 1, tsl],
                                        start=(i == 0),
                                        stop=(i == nfl - 1),
                                    )
                            # y_acc = y8 * Y8_DESCALE + y
                            nc.vector.tensor_scalar_mul(
                                y_acc[:, do, tsl], y8_ps[:, :tn], Y8_DESCALE
                            )
                            nc.vector.tensor_add(
                                y_acc[:, do, tsl], y_acc[:, do, tsl],
                                y_ps[:, :tn],
                            )
                        else:
                            for fl in range(nfl):
                                nc.tensor.matmul(
                                    y_ps[:, :tn],
                                    wd_t[:, fl : fl + 1, :],
                                    h[:, fl0 + fl : fl0 + fl + 1, tsl],
                                    start=(fl == 0),
                                    stop=(fl == nfl - 1),
                                )
                            nc.vector.tensor_add(
                                y_acc[:, do, tsl], y_acc[:, do, tsl],
                                y_ps[:, :tn],
                            )
                        if q == NQ - 1:
                            y_sb = sp.tile([P, TC], dt.float32, tag="yo")
                            if do == DT - 1 and c == len(sizes) - 1:
                                # last output tile: halve the scale+store so
                                # the DVE work pipelines against the DMA
                                # instead of serializing on the kernel tail
                                hh = tn // 2
                                for s0, s1 in ((0, hh), (hh, tn)):
                                    ssl = slice(t0 + s0, t0 + s1)
                                    nc.vector.tensor_mul(
                                        y_sb[:, s0:s1],
                                        y_acc[:, do, ssl], tw[:, ssl],
                                    )
                                    nc.sync.dma_start(
                                        out=y_d[do, :, ssl],
                                        in_=y_sb[:, s0:s1],
                                    )
                            else:
                                nc.vector.tensor_mul(
                                    y_sb[:, :tn], y_acc[:, do, tsl],
                                    tw[:, tsl],
                                )
                                nc.sync.dma_start(
                                    out=y_d[do, :, tsl], in_=y_sb[:, :tn]
                                )

    nc.compile()
    return nc


def _tile_w_in(w_t):
    """[D, FF] (already transposed) -> [FF/P, P, D/P, P] contiguous bf16."""
    # out[f, p, ki, m] = w_t[ki*128+p, f*128+m]
    r = w_t.reshape(KT, P, FT, P).transpose(2, 1, 0, 3)
    return np.ascontiguousarray(r, dtype=ml_dtypes.bfloat16)


def _tile_w_down(w):
    """w_down [D, FF] -> [D/P, NQ, P, FQ, P] contiguous bf16.

    out[do, q, p, fl, m] = w[do*128+m, (q*FQ+fl)*128+p]
    """
    r = w.reshape(DT, P, NQ, FQ, P).transpose(0, 2, 4, 3, 1)
    return np.ascontiguousarray(r, dtype=ml_dtypes.bfloat16)


def _tile_w_down8(w):
    """First NF8 f-tiles of w_down [D, FF] -> [DT, P, NP8, 2, P] e4m3.

    out[do, p, j, i, m] = w[do*128+m, (2j+i)*128+p] * WD8_SCALE
    """
    r = w[:, : NF8 * P].reshape(DT, P, NP8, 2, P).transpose(0, 4, 2, 3, 1)
    r = np.clip(r.astype(np.float64) * WD8_SCALE, -240, 240).astype(np.float32)
    return np.ascontiguousarray(r.astype(ml_dtypes.float8_e4m3))


def kernel(hidden_states, gate_w, w_gate, w_up, w_down):
    from concourse.bass_utils import run_bass_kernel_spmd

    hidden_states = np.asarray(hidden_states)
    gate_w = np.asarray(gate_w)
    w_gate = np.asarray(w_gate)
    w_up = np.asarray(w_up)
    w_down = np.asarray(w_down)

    x = hidden_states.reshape(T, D)

    # --- router (tiny: T x E) on host, fp64 for stable argmax ---
    logits = x.astype(np.float64) @ gate_w.astype(np.float64).T  # [T, E]
    m = logits.max(axis=1, keepdims=True)
    p = np.exp(logits - m)
    p /= p.sum(axis=1, keepdims=True)
    sel = np.argmax(p, axis=1)  # [T]
    top_w = p[np.arange(T), sel].astype(np.float32)  # [T]

    # --- dispatch: split each expert's tokens across its cores ---
    idx_e = [np.nonzero(sel == e)[0] for e in range(E)]
    t0, t1 = len(idx_e[0]), len(idx_e[1])

    # Capacity-factor-1.0 dispatch: fixed per-core capacity C = T/8 keeps the
    # device program perfectly balanced (2x512 PSUM chunks); the few overflow
    # tokens of an over-loaded expert are computed exactly on the host during
    # combine. Falls back to elastic capacity if routing is badly skewed.
    CAP = T // N_CORES  # 1024
    best = None
    for n0 in range(1, N_CORES):
        n1 = N_CORES - n0
        ovf = max(0, t0 - n0 * CAP) + max(0, t1 - n1 * CAP)
        if best is None or ovf < best[0]:
            best = (ovf, n0)
    if best[0] <= 512:
        C = CAP
        n0 = best[1]
    else:
        best = None
        for n0 in range(1, N_CORES):
            n1 = N_CORES - n0
            load = max(math.ceil(t0 / n0) if t0 else 0,
                       math.ceil(t1 / n1) if t1 else 0)
            if best is None or load < best[0]:
                best = (load, n0)
        # multiple of 16 so the DoubleRow moving-AP pair stride (C bytes)
        # satisfies the step%16 constraint
        C = max(P, ((best[0] + 15) // 16) * 16)
        n0 = best[1]
    cores_per_exp = [n0, N_CORES - n0]

    core_expert = []
    core_tok = []
    host_ids = []
    host_exp = []
    for e in range(E):
        ids = idx_e[e]
        nce = cores_per_exp[e]
        if len(ids) > nce * C:
            host_ids.append(ids[nce * C :])
            host_exp.append(e)
            ids = ids[: nce * C]
        per = math.ceil(len(ids) / nce) if len(ids) else 0
        for j in range(nce):
            core_expert.append(e)
            core_tok.append(ids[j * per : (j + 1) * per])

    nc = _nc_cache.get(C)
    if nc is None:
        nc = _build_nc(C)
        _nc_cache[C] = nc

    # --- per-expert weight tiling (shared across that expert's cores) ---
    F8 = NF8 * P
    wg_tiled = [_tile_w_in(w_gate[e].T) for e in range(E)]
    wu_tiled = []
    for e in range(E):
        wu_e = w_up[e].copy()
        wu_e[:F8] *= H8_SCALE  # fold the h8 e4m3 scale into up_proj
        wu_tiled.append(_tile_w_in(wu_e.T))
    wd_tiled = [_tile_w_down(w_down[e]) for e in range(E)]
    wd8_tiled = [_tile_w_down8(w_down[e]) for e in range(E)]

    in_maps = []
    for c in range(N_CORES):
        e = core_expert[c]
        ids = core_tok[c]
        n = len(ids)
        xt = np.zeros((P, KT, C), dtype=ml_dtypes.bfloat16)
        if n:
            # xc [n, D] -> [ki, p, t] -> [p, ki, t]
            xc = x[ids].astype(ml_dtypes.bfloat16)
            xt[:, :, :n] = xc.T.reshape(KT, P, n).transpose(1, 0, 2)
        tw = np.zeros((P, C), dtype=np.float32)
        if n:
            tw[:, :n] = top_w[ids][None, :]
        in_maps.append({
            "xt": xt,
            "wg": wg_tiled[e],
            "wu": wu_tiled[e],
            "wd": wd_tiled[e],
            "wd8": wd8_tiled[e],
            "tw": tw,
        })

    res = run_bass_kernel_spmd(nc, in_maps, list(range(N_CORES)))
    global LAST
    LAST = res

    # --- combine ---
    out = np.zeros((T, D), dtype=np.float32)
    for c in range(N_CORES):
        ids = core_tok[c]
        n = len(ids)
        if not n:
            continue
        y = res.results[c]["y"]  # [DT, P, C]
        out[ids] = y.reshape(D, C)[:, :n].T
    # host-side exact MLP for capacity-overflow tokens
    for e, ids in zip(host_exp, host_ids):
        xe = x[ids]
        g = xe @ w_gate[e].T
        u = xe @ w_up[e].T
        h = (g * 0.5 * (1.0 + np.tanh(0.5 * g))) * u
        out[ids] = (h @ w_down[e].T) * top_w[ids][:, None]
    return out.reshape(B, S, D)



# revision 28
# speedup vs baseline: 1.0057x; 1.0054x over previous
"""Top-1 MoE layer (Mistral MLP experts, E=2) on 8 Trainium2 cores.

Strategy (expert-parallel + data-parallel, host does dispatch/combine):
  - Host computes the tiny router (T x E logits, softmax, argmax) in fp64,
    sorts token indices by assigned expert, and splits each expert's tokens
    across that expert's cores with a fixed capacity-factor-1.0 budget
    (C = T/8 = 1024 tokens/core, i.e. two full 512-column PSUM chunks).
    The few tokens an over-loaded expert can't fit (59 here) are computed
    exactly on the host during combine.
  - Each core receives: its packed tokens (transposed, bf16, k-tiled), its
    expert's weights pre-tiled so every device DMA is fully contiguous, and
    the routing weight per token (replicated across partitions).
  - Device kernel per core (bf16 matmuls, fp32 PSUM accumulation): FF is
    processed in quarters so each weight byte is streamed from HBM exactly
    once; h = silu(x@Wg^T) * (x@Wu^T) for a quarter stays in SBUF, partial
    down-projections accumulate into an SBUF fp32 y buffer, and the final
    quarter fuses the per-token routing-weight scale. No collectives.
  - The first NF8=14 f-tiles of the down projection run as e4m3 DoubleRow
    matmuls (2x PE rate): h for those channels is stored e4m3 (scale 16
    folded into up_proj on host), wd8 is e4m3 x512, and the 2^-13 descale
    happens in the y-accumulate. Measured rel_err 1.79e-2 vs the 2e-2
    gate (all-bf16 is 4.1e-3; error budget deliberately spent for speed).
  - Startup: f0 weights + x chunks interleaved in fine pieces on one DMA
    queue in consumption order, so the first matmul issues ~4us after the
    DMA engines spin up and the PE never starves during the ramp.
  - Host scatters per-core outputs back to token order.
"""

import math

import numpy as np
import ml_dtypes


def _ensure_ntff_hook():
    """bass_utils' trace path imports antenv.axon_hooks; some images lack
    it. Register the equivalent ctypes-based hook if (and only if) the
    real module is missing, so tracing degrades gracefully either way."""
    try:
        import antenv.axon_hooks  # noqa: F401
        return
    except ImportError:
        pass
    try:
        import sys
        import types

        import antenv
        from trn_agent_boot.trn_boot import _ntff_profile_via_ctypes

        hook = _ntff_profile_via_ctypes("/opt/axon/libaxon_pjrt.so")
        mod = types.ModuleType("antenv.axon_hooks")
        mod.get_axon_ntff_profile_hook = lambda: hook
        mod.set_axon_ntff_profile_hook = lambda h: None
        sys.modules["antenv.axon_hooks"] = mod
        antenv.axon_hooks = mod
    except Exception:
        pass


_ensure_ntff_hook()

B, S, D, FF, E = 4, 2048, 2048, 8192, 2
T = B * S
P = 128
KT = D // P   # 16 contraction tiles for gate/up
FT = FF // P  # 64 f tiles
DT = D // P   # 16 output-row tiles for down
NQ = 4        # FF quarters
FQ = FT // NQ  # 16 f tiles per quarter
N_CORES = 8
MAX_N = 512   # matmul free-dim / PSUM bank limit (fp32 out)
# First NF8 f-tiles of the down projection run as e4m3 DoubleRow matmuls
# (2x PE rate). Error budget: measured rel_err 1.7e-2 at NF8=12 vs the
# 2e-2 gate (bf16 baseline 4.1e-3); h is stored e4m3 scaled by 16 (wu
# host-fold; max |16h| = 179 < 240 so no e4m3 overflow on this data),
# wd8 scaled by 512, descale 2^-13 fused into the combine.
NF8 = 16
NP8 = NF8 // 2  # DoubleRow pairs
H8_SCALE = 16.0
WD8_SCALE = 512.0
Y8_DESCALE = 1.0 / (H8_SCALE * WD8_SCALE)

_nc_cache: dict[int, object] = {}

# Last BassKernelResults (for external profiling harnesses).
LAST = None


def _chunks(C):
    n = max(1, math.ceil(C / MAX_N))
    tc = min(MAX_N, ((C + n - 1) // n + 7) // 8 * 8)
    sizes = []
    left = C
    for _ in range(n):
        sizes.append(min(tc, left))
        left -= sizes[-1]
    assert sum(sizes) == C and all(0 < s <= MAX_N for s in sizes)
    return sizes


def _build_nc(C: int):
    """Build + compile the single-core Bass program (SPMD across 8 cores).

    C = per-core token capacity (multiple of 8).
    """
    import concourse.mybir as mybir
    import concourse.tile as tile
    from concourse import bacc

    dt = mybir.dt
    nc = bacc.Bacc("TRN2", target_bir_lowering=False, debug=False,
                   num_devices=N_CORES)

    # xt[p, ki, t] = x_packed[t, ki*128 + p]
    xt_d = nc.dram_tensor("xt", [P, KT, C], dt.bfloat16, kind="ExternalInput")
    # wg[f, p, ki, m] = w_gate[f*128+m, ki*128+p] (one expert)
    wg_d = nc.dram_tensor("wg", [FT, P, KT, P], dt.bfloat16, kind="ExternalInput")
    wu_d = nc.dram_tensor("wu", [FT, P, KT, P], dt.bfloat16, kind="ExternalInput")
    # wd[do, q, p, fl, m] = w_down[do*128+m, (q*FQ+fl)*128+p]
    wd_d = nc.dram_tensor("wd", [DT, NQ, P, FQ, P], dt.bfloat16,
                          kind="ExternalInput")
    # fp8 down-proj tiles for the first NF8 f-tiles (DoubleRow pairs):
    # wd8[do, p, j, i, m] = w_down[do*128+m, (2j+i)*128+p] * WD8_SCALE (e4m3)
    wd8_d = nc.dram_tensor("wd8", [DT, P, NP8, 2, P], dt.float8e4,
                           kind="ExternalInput")
    # tw[p, t] = routing weight of token t (same for all p)
    tw_d = nc.dram_tensor("tw", [P, C], dt.float32, kind="ExternalInput")
    # y[do, m, t] = out_packed[t, do*128+m]
    y_d = nc.dram_tensor("y", [DT, P, C], dt.float32, kind="ExternalOutput")

    sizes = _chunks(C)
    starts = [sum(sizes[:i]) for i in range(len(sizes))]
    TC = sizes[0]
    # at very large C (heavily skewed routing) the resident x/h/y buffers
    # leave less SBUF headroom — shrink the weight-stream double-buffering
    wbufs = 3 if C <= 1100 else 2

    with tile.TileContext(nc) as tc:
        with (
            tc.tile_pool(name="persist", bufs=1) as pp,
            tc.tile_pool(name="wgwu", bufs=wbufs) as wp,
            tc.tile_pool(name="wdp", bufs=2) as dp,
            tc.tile_pool(name="hbuf", bufs=1) as hp,
            tc.tile_pool(name="stage", bufs=2) as sp,
            tc.tile_pool(name="psum", bufs=2, space="PSUM") as psp,
        ):
            xt = pp.tile([P, KT, C], dt.bfloat16)
            tw = pp.tile([P, C], dt.float32)
            h = hp.tile([P, FQ, C], dt.bfloat16)
            # e4m3 h for the fp8 f-tiles (quarter 0, fl < NF8); holds
            # H8_SCALE*h because wu rows for those channels are host-scaled.
            h8 = hp.tile([P, NF8, C], dt.float8e4)
            y_acc = pp.tile([P, DT, C], dt.float32)

            # Startup critical path: the first matmul group (f=0, chunk 0)
            # needs wg[0] + x chunk 0. Interleave those in half-KT pieces at
            # the head of the sync queue for fine-grained dependencies, then
            # stream the remaining x chunks in need-order on the same queue
            # (no bandwidth race from a second queue during ramp-up).
            # HAM prewarm: the first real matmul is DMA-gated until ~12us,
            # and the sparse ramp keeps the PE clock-gate at K=4/8 (half
            # clock) for its first ~10us. A burst of dummy matmuls on a
            # zeroed tile during the pre-DMA dead time opens the gate
            # before real work arrives. Output lands in y_acc[:,0] which
            # quarter 0 fully overwrites before any read.
            wz = pp.tile([P, TC], dt.bfloat16)
            nc.gpsimd.memset(wz[:], 0)
            warm_ps = psp.tile([P, TC], dt.float32, tag="y8")
            for _ in range(10):
                nc.tensor.matmul(
                    warm_ps[:], wz[:, :P], wz[:], start=True, stop=True
                )
            nc.vector.tensor_copy(y_acc[:, 0, 0:TC], warm_ps[:])

            wg0 = wp.tile([P, KT, P], dt.bfloat16, tag="wg")
            wu0 = wp.tile([P, KT, P], dt.bfloat16, tag="wu")
            c0n = sizes[0]
            # finer pieces early (DMA engines still ramping), coarser later
            bounds = [0, 2, 4, 6, 8, 12, 16]
            for lo, hi in zip(bounds, bounds[1:]):
                ks = slice(lo, hi)
                nc.sync.dma_start(out=wg0[:, ks], in_=wg_d[0, :, ks])
                nc.sync.dma_start(out=xt[:, ks, 0:c0n], in_=xt_d[:, ks, 0:c0n])
            nc.sync.dma_start(out=wu0[:], in_=wu_d[0])
            # Later x chunks ride the scalar HWDGE queue: per-queue
            # descriptor throughput (~250GB/s) is the early-ramp limit,
            # so a second queue in parallel roughly doubles the feed rate
            # while the sync queue carries chunk 0 + the weight stream.
            for c in range(1, len(sizes)):
                t0, tn = starts[c], sizes[c]
                half = KT // 2
                nc.scalar.dma_start(
                    out=xt[:, :half, t0 : t0 + tn],
                    in_=xt_d[:, :half, t0 : t0 + tn],
                )
                nc.scalar.dma_start(
                    out=xt[:, half:, t0 : t0 + tn],
                    in_=xt_d[:, half:, t0 : t0 + tn],
                )
            nc.scalar.dma_start(out=tw[:], in_=tw_d[:])

            for q in range(NQ):
                # phase A: h[fl] = silu(x @ Wg^T) * (x @ Wu^T) for this
                # quarter. Chunk-outer so chunk c of x is first consumed
                # ~c*5.7us into the stream (relaxes the startup DMA).
                for fl in range(FQ):
                    f = q * FQ + fl
                    if f == 0:
                        wg_t, wu_t = wg0, wu0
                    else:
                        wg_t = wp.tile([P, KT, P], dt.bfloat16, tag="wg")
                        nc.sync.dma_start(out=wg_t[:], in_=wg_d[f])
                        wu_t = wp.tile([P, KT, P], dt.bfloat16, tag="wu")
                        nc.sync.dma_start(out=wu_t[:], in_=wu_d[f])
                    for c, (t0, tn) in enumerate(zip(starts, sizes)):
                        tsl = slice(t0, t0 + tn)
                        g_ps = psp.tile([P, TC], dt.float32, tag="g")
                        u_ps = psp.tile([P, TC], dt.float32, tag="u")
                        for ki in range(KT):
                            nc.tensor.matmul(
                                g_ps[:, :tn],
                                wg_t[:, ki : ki + 1, :],
                                xt[:, ki : ki + 1, tsl],
                                start=(ki == 0),
                                stop=(ki == KT - 1),
                            )
                        for ki in range(KT):
                            nc.tensor.matmul(
                                u_ps[:, :tn],
                                wu_t[:, ki : ki + 1, :],
                                xt[:, ki : ki + 1, tsl],
                                start=(ki == 0),
                                stop=(ki == KT - 1),
                            )
                        sg = sp.tile([P, TC], dt.float32, tag="sg")
                        nc.scalar.activation(
                            sg[:, :tn], g_ps[:, :tn],
                            mybir.ActivationFunctionType.Silu,
                        )
                        if q == 0 and fl < NF8:
                            # store H8_SCALE*h as e4m3 for the fp8 down tiles
                            nc.vector.tensor_mul(
                                h8[:, fl, tsl], sg[:, :tn], u_ps[:, :tn]
                            )
                        else:
                            nc.vector.tensor_mul(
                                h[:, fl, tsl], sg[:, :tn], u_ps[:, :tn]
                            )
                # phase B: y_acc += h @ Wd^T (this quarter's partial)
                for do in range(DT):
                    if q == 0:
                        # fl < NF8 handled by fp8 DoubleRow; bf16 remainder
                        if FQ > NF8:
                            wd_t = dp.tile([P, FQ - NF8, P], dt.bfloat16,
                                           tag="wd")
                            nc.sync.dma_start(out=wd_t[:],
                                              in_=wd_d[do, q, :, NF8:])
                        wd8_t = dp.tile([P, NP8, 2, P], dt.float8e4,
                                        tag="wd8")
                        nc.sync.dma_start(out=wd8_t[:], in_=wd8_d[do])
                    else:
                        wd_t = dp.tile([P, FQ, P], dt.bfloat16, tag="wd")
                        nc.sync.dma_start(out=wd_t[:], in_=wd_d[do, q])
                    for c, (t0, tn) in enumerate(zip(starts, sizes)):
                        tsl = slice(t0, t0 + tn)
                        nfl = (FQ - NF8) if q == 0 else FQ
                        fl0 = NF8 if q == 0 else 0
                        y_ps = None
                        if nfl > 0:
                            y_ps = psp.tile([P, TC], dt.float32, tag="y")
                        if q == 0:
                            # DoubleRow LDWEIGHTS (256 cols, no FWL) barely
                            # exceeds one 512-col matmul window, so back-to-
                            # back DR matmuls slip ~2x. Interleave the two
                            # cheap-LDW bf16 matmuls between DR pairs to
                            # give the weight port slack. The two PSUM
                            # accumulation groups are independent banks.
                            y8_ps = psp.tile([P, TC], dt.float32, tag="y8")
                            seq = []
                            nb = 0
                            for j in range(NP8):
                                seq.append(("dr", j))
                                if j % 2 == 1 and nb < nfl:
                                    seq.append(("bf", nb))
                                    nb += 1
                            while nb < nfl:
                                seq.append(("bf", nb))
                                nb += 1
                            for kind, i in seq:
                                if kind == "dr":
                                    nc.tensor.matmul(
                                        y8_ps[:, :tn],
                                        wd8_t[:, i],
                                        h8[:, 2 * i : 2 * i + 2, tsl],
                                        start=(i == 0),
                                        stop=(i == NP8 - 1),
                                        perf_mode=(
                                            mybir.MatmulPerfMode.DoubleRow
                                        ),
                                    )
                                else:
                                    nc.tensor.matmul(
                                        y_ps[:, :tn],
                                        wd_t[:, i : i + 1, :],
                                        h[:, fl0 + i : fl0 + i + 1, tsl],
                                        start=(i == 0),
                                        stop=(i == nfl - 1),
                                    )
                            # y_acc = y8 * Y8_DESCALE (+ y if any bf16 part)
                            nc.vector.tensor_scalar_mul(
                                y_acc[:, do, tsl], y8_ps[:, :tn], Y8_DESCALE
                            )
                            if nfl > 0:
                                nc.vector.tensor_add(
                                    y_acc[:, do, tsl], y_acc[:, do, tsl],
                                    y_ps[:, :tn],
                                )
                        else:
                            for fl in range(nfl):
                                nc.tensor.matmul(
                                    y_ps[:, :tn],
                                    wd_t[:, fl : fl + 1, :],
                                    h[:, fl0 + fl : fl0 + fl + 1, tsl],
                                    start=(fl == 0),
                                    stop=(fl == nfl - 1),
                                )
                            nc.vector.tensor_add(
                                y_acc[:, do, tsl], y_acc[:, do, tsl],
                                y_ps[:, :tn],
                            )
                        if q == NQ - 1:
                            y_sb = sp.tile([P, TC], dt.float32, tag="yo")
                            if do == DT - 1 and c == len(sizes) - 1:
                                # last output tile: halve the scale+store so
                                # the DVE work pipelines against the DMA
                                # instead of serializing on the kernel tail
                                hh = tn // 2
                                for s0, s1 in ((0, hh), (hh, tn)):
                                    ssl = slice(t0 + s0, t0 + s1)
                                    nc.vector.tensor_mul(
                                        y_sb[:, s0:s1],
                                        y_acc[:, do, ssl], tw[:, ssl],
                                    )
                                    nc.sync.dma_start(
                                        out=y_d[do, :, ssl],
                                        in_=y_sb[:, s0:s1],
                                    )
                            else:
                                nc.vector.tensor_mul(
                                    y_sb[:, :tn], y_acc[:, do, tsl],
                                    tw[:, tsl],
                                )
                                nc.sync.dma_start(
                                    out=y_d[do, :, tsl], in_=y_sb[:, :tn]
                                )

    nc.compile()
    return nc


def _tile_w_in(w_t):
    """[D, FF] (already transposed) -> [FF/P, P, D/P, P] contiguous bf16."""
    # out[f, p, ki, m] = w_t[ki*128+p, f*128+m]
    r = w_t.reshape(KT, P, FT, P).transpose(2, 1, 0, 3)
    return np.ascontiguousarray(r, dtype=ml_dtypes.bfloat16)


def _tile_w_down(w):
    """w_down [D, FF] -> [D/P, NQ, P, FQ, P] contiguous bf16.

    out[do, q, p, fl, m] = w[do*128+m, (q*FQ+fl)*128+p]
    """
    r = w.reshape(DT, P, NQ, FQ, P).transpose(0, 2, 4, 3, 1)
    return np.ascontiguousarray(r, dtype=ml_dtypes.bfloat16)


def _tile_w_down8(w):
    """First NF8 f-tiles of w_down [D, FF] -> [DT, P, NP8, 2, P] e4m3.

    out[do, p, j, i, m] = w[do*128+m, (2j+i)*128+p] * WD8_SCALE
    """
    r = w[:, : NF8 * P].reshape(DT, P, NP8, 2, P).transpose(0, 4, 2, 3, 1)
    r = np.clip(r.astype(np.float64) * WD8_SCALE, -240, 240).astype(np.float32)
    return np.ascontiguousarray(r.astype(ml_dtypes.float8_e4m3))


def kernel(hidden_states, gate_w, w_gate, w_up, w_down):
    from concourse.bass_utils import run_bass_kernel_spmd

    hidden_states = np.asarray(hidden_states)
    gate_w = np.asarray(gate_w)
    w_gate = np.asarray(w_gate)
    w_up = np.asarray(w_up)
    w_down = np.asarray(w_down)

    x = hidden_states.reshape(T, D)

    # --- router (tiny: T x E) on host, fp64 for stable argmax ---
    logits = x.astype(np.float64) @ gate_w.astype(np.float64).T  # [T, E]
    m = logits.max(axis=1, keepdims=True)
    p = np.exp(logits - m)
    p /= p.sum(axis=1, keepdims=True)
    sel = np.argmax(p, axis=1)  # [T]
    top_w = p[np.arange(T), sel].astype(np.float32)  # [T]

    # --- dispatch: split each expert's tokens across its cores ---
    idx_e = [np.nonzero(sel == e)[0] for e in range(E)]
    t0, t1 = len(idx_e[0]), len(idx_e[1])

    # Capacity-factor-1.0 dispatch: fixed per-core capacity C = T/8 keeps the
    # device program perfectly balanced (2x512 PSUM chunks); the few overflow
    # tokens of an over-loaded expert are computed exactly on the host during
    # combine. Falls back to elastic capacity if routing is badly skewed.
    CAP = T // N_CORES  # 1024
    best = None
    for n0 in range(1, N_CORES):
        n1 = N_CORES - n0
        ovf = max(0, t0 - n0 * CAP) + max(0, t1 - n1 * CAP)
        if best is None or ovf < best[0]:
            best = (ovf, n0)
    if best[0] <= 512:
        C = CAP
        n0 = best[1]
    else:
        best = None
        for n0 in range(1, N_CORES):
            n1 = N_CORES - n0
            load = max(math.ceil(t0 / n0) if t0 else 0,
                       math.ceil(t1 / n1) if t1 else 0)
            if best is None or load < best[0]:
                best = (load, n0)
        # multiple of 16 so the DoubleRow moving-AP pair stride (C bytes)
        # satisfies the step%16 constraint
        C = max(P, ((best[0] + 15) // 16) * 16)
        n0 = best[1]
    cores_per_exp = [n0, N_CORES - n0]

    core_expert = []
    core_tok = []
    host_ids = []
    host_exp = []
    for e in range(E):
        ids = idx_e[e]
        nce = cores_per_exp[e]
        if len(ids) > nce * C:
            host_ids.append(ids[nce * C :])
            host_exp.append(e)
            ids = ids[: nce * C]
        per = math.ceil(len(ids) / nce) if len(ids) else 0
        for j in range(nce):
            core_expert.append(e)
            core_tok.append(ids[j * per : (j + 1) * per])

    nc = _nc_cache.get(C)
    if nc is None:
        nc = _build_nc(C)
        _nc_cache[C] = nc

    # --- per-expert weight tiling (shared across that expert's cores) ---
    F8 = NF8 * P
    wg_tiled = [_tile_w_in(w_gate[e].T) for e in range(E)]
    wu_tiled = []
    for e in range(E):
        wu_e = w_up[e].copy()
        wu_e[:F8] *= H8_SCALE  # fold the h8 e4m3 scale into up_proj
        wu_tiled.append(_tile_w_in(wu_e.T))
    wd_tiled = [_tile_w_down(w_down[e]) for e in range(E)]
    wd8_tiled = [_tile_w_down8(w_down[e]) for e in range(E)]

    in_maps = []
    for c in range(N_CORES):
        e = core_expert[c]
        ids = core_tok[c]
        n = len(ids)
        xt = np.zeros((P, KT, C), dtype=ml_dtypes.bfloat16)
        if n:
            # xc [n, D] -> [ki, p, t] -> [p, ki, t]
            xc = x[ids].astype(ml_dtypes.bfloat16)
            xt[:, :, :n] = xc.T.reshape(KT, P, n).transpose(1, 0, 2)
        tw = np.zeros((P, C), dtype=np.float32)
        if n:
            tw[:, :n] = top_w[ids][None, :]
        in_maps.append({
            "xt": xt,
            "wg": wg_tiled[e],
            "wu": wu_tiled[e],
            "wd": wd_tiled[e],
            "wd8": wd8_tiled[e],
            "tw": tw,
        })

    res = run_bass_kernel_spmd(nc, in_maps, list(range(N_CORES)))
    global LAST
    LAST = res

    # --- combine ---
    out = np.zeros((T, D), dtype=np.float32)
    for c in range(N_CORES):
        ids = core_tok[c]
        n = len(ids)
        if not n:
            continue
        y = res.results[c]["y"]  # [DT, P, C]
        out[ids] = y.reshape(D, C)[:, :n].T
    # host-side exact MLP for capacity-overflow tokens
    for e, ids in zip(host_exp, host_ids):
        xe = x[ids]
        g = xe @ w_gate[e].T
        u = xe @ w_up[e].T
        h = (g * 0.5 * (1.0 + np.tanh(0.5 * g))) * u
        out[ids] = (h @ w_down[e].T) * top_w[ids][:, None]
    return out.reshape(B, S, D)

